# revision 17
# baseline (speedup 1.0000x reference)
"""Trainium2 Bass kernel for the DJconv hypergraph message-passing layer.

Reference computation (per full input):
    gram = H.T @ H                              [E, E]
    Hu   = concat([H, H @ gram], 1) >= 0.5      [N, 2E] binary
    dv   = Hu.sum(1);  inv = rsqrt(dv) (0 where dv==0)
    out  = ((1 + inv)[:, None] * U) @ weight + bias

Sharding: rows (nodes) split across 8 NeuronCores; the [E, E] gram is
all-reduced on device; weight/bias replicated.
"""

import numpy as np
import ml_dtypes

import concourse.bass as bass
import concourse.tile as tile
from concourse import bacc, mybir
from concourse.bass_utils import run_bass_kernel_spmd

F32 = mybir.dt.float32
F32R = mybir.dt.float32r
BF16 = mybir.dt.bfloat16

N_FULL, E, IN_C, OUT_C = 131072, 256, 128, 256
NCORES = 8
ROWS = N_FULL // NCORES          # 16384 rows per core
P = 128


def build_program(rows=ROWS, ncores=NCORES):
    """Build + compile the SPMD single-core program (same NEFF on all cores)."""
    assert rows % 512 == 0
    nt = rows // P          # node tiles per core
    ns = nt // 4            # super tiles (4 node tiles each)

    nc = bacc.Bacc("TRN2", target_bir_lowering=False, debug=False,
                   num_devices=ncores)

    H = nc.dram_tensor("H", [rows, E], F32, kind="ExternalInput").ap()
    U = nc.dram_tensor("U", [rows, IN_C], F32, kind="ExternalInput").ap()
    W = nc.dram_tensor("W", [IN_C, OUT_C], F32, kind="ExternalInput").ap()
    BIASB = nc.dram_tensor("BIASB", [P, OUT_C], F32, kind="ExternalInput").ap()
    ID16 = nc.dram_tensor("ID16", [P, P], BF16, kind="ExternalInput").ap()
    ID32 = nc.dram_tensor("ID32", [P, P], F32, kind="ExternalInput").ap()
    OUT = nc.dram_tensor("OUT", [rows, OUT_C], F32, kind="ExternalOutput").ap()

    # super-tile views: node (s*512 + j*128 + p)
    H_r = H.rearrange("(s j p) e -> s p j e", j=4, p=P)
    U_r = U.rearrange("(s j p) c -> s p j c", j=4, p=P)
    OUT_r = OUT.rearrange("(s j p) o -> s p j o", j=4, p=P)

    with tile.TileContext(nc) as tc:
        _body(tc, nt, ns, H_r, U_r, OUT_r, W, BIASB, ID16, ID32)

    nc.compile()
    return nc


def _body(tc, nt, ns, H_r, U_r, OUT_r, W, BIASB, ID16, ID32):
    nc = tc.nc
    Add = mybir.AluOpType.add
    Mult = mybir.AluOpType.mult
    IsGe = mybir.AluOpType.is_ge
    AF = mybir.ActivationFunctionType

    import contextlib
    ctx = contextlib.ExitStack()
    with ctx:
        const = ctx.enter_context(tc.tile_pool(name="const", bufs=1))
        hpool = ctx.enter_context(tc.tile_pool(name="hload", bufs=3))
        htst = ctx.enter_context(tc.tile_pool(name="htstore", bufs=1))
        work = ctx.enter_context(tc.tile_pool(name="work", bufs=1))
        upool = ctx.enter_context(tc.tile_pool(name="uload", bufs=3))
        mtpool = ctx.enter_context(tc.tile_pool(name="mut", bufs=2))
        opool = ctx.enter_context(tc.tile_pool(name="ost", bufs=2))
        scr = ctx.enter_context(tc.tile_pool(name="scratch", bufs=2))
        dram = ctx.enter_context(tc.tile_pool(name="dram", bufs=1, space="DRAM"))

        # ---- constants ----
        id16 = const.tile([P, P], BF16)
        nc.sync.dma_start(id16[:], ID16[:])
        id32 = const.tile([P, P], F32)
        nc.sync.dma_start(id32[:], ID32[:])
        w_sb = const.tile([IN_C, OUT_C], F32)
        nc.sync.dma_start(w_sb[:], W[:])
        bias_b = const.tile([P, OUT_C], F32)
        nc.sync.dma_start(bias_b[:], BIASB[:])
        neghalf = const.tile([P, 1], F32)
        nc.vector.memset(neghalf[:], -0.5)

        # persistent H^T (feature-major H), two row blocks of 128 edges
        HT0 = htst.tile([P, nt * P], BF16, tag="ht0")
        HT1 = htst.tile([P, nt * P], BF16, tag="ht1")
        # all of H stays resident (bf16) so transposes can fill the collective window
        HALL = htst.tile([P, ns, 4, E], BF16, tag="hall")

        # ---- phase A: stream H, gram triangle ----
        with tc.tile_pool(name="psA", bufs=1, space="PSUM") as psA:
            gA = psA.tile([P, E], F32, tag="gA")      # gram rows 0:128, all cols
            gB = psA.tile([P, P], F32, tag="gB")      # gram rows 128:256, cols 128:256

            for s in range(ns):
                nc.gpsimd.dma_start(HALL[:, s, :, :], H_r[s])  # fp32 -> bf16 cast
                for j in range(4):
                    k = 4 * s + j
                    first, last = (k == 0), (k == nt - 1)
                    nc.tensor.matmul(gA[:], HALL[:, s, j, 0:P], HALL[:, s, j, :],
                                     start=first, stop=last)
                    nc.tensor.matmul(gB[:], HALL[:, s, j, P:E], HALL[:, s, j, P:E],
                                     start=first, stop=last)

            # ---- gram all-gather + local sum ----
            gcat = work.tile([P, E + P], F32, tag="gcat")
            nc.vector.tensor_copy(gcat[:, 0:E], gA[:])
            nc.vector.tensor_copy(gcat[:, E:E + P], gB[:])
        cc_in = dram.tile([P, E + P], F32)
        cc_out = dram.tile([NCORES * P, E + P], F32)
        nc.sync.dma_start(cc_in[:], gcat[:])
        nc.gpsimd.collective_compute(
            "AllGather", mybir.AluOpType.bypass,
            replica_groups=[list(range(NCORES))],
            ins=[cc_in.opt()],
            outs=[cc_out.opt()],
        )
        gparts = work.tile([P, NCORES, E + P], F32, tag="gparts")
        cc_out_r = cc_out[:].rearrange("(r p) f -> p r f", p=P)
        nc.sync.dma_start(gparts[:], cc_out_r)

        # ---- H^T transposes (overlap the collective) ----
        with tc.tile_pool(name="psT", bufs=2, space="PSUM") as psT:
            for s in range(ns):
                pt0 = psT.tile([P, 4 * P], BF16, tag="t0")
                pt1 = psT.tile([P, 4 * P], BF16, tag="t1")
                for j in range(4):
                    nc.tensor.transpose(pt0[:, j * P:(j + 1) * P],
                                        HALL[:, s, j, 0:P], id16[:])
                    nc.tensor.transpose(pt1[:, j * P:(j + 1) * P],
                                        HALL[:, s, j, P:E], id16[:])
                sl = slice(s * 4 * P, (s + 1) * 4 * P)
                nc.vector.tensor_copy(HT0[:, sl], pt0[:])
                nc.scalar.copy(HT1[:, sl], pt1[:])

        # tree-sum the 8 gathered gram partials: [P, 8, F] -> [P, F]
        gsum = work.tile([P, E + P], F32, tag="gsum")
        g4 = work.tile([P, 4, E + P], F32, tag="g4")
        nc.vector.tensor_tensor(g4[:], gparts[:, 0:4, :], gparts[:, 4:8, :], op=Add)
        g2 = work.tile([P, 2, E + P], F32, tag="g2")
        nc.vector.tensor_tensor(g2[:], g4[:, 0:2, :], g4[:, 2:4, :], op=Add)
        nc.vector.tensor_tensor(gsum[:], g2[:, 0, :], g2[:, 1, :], op=Add)

        # ---- phase B: HG tiles, threshold counts ----
        dvS = work.tile([P, nt], F32, tag="dvS")   # per-tile accumulated counts
        dvH = work.tile([P, nt], F32, tag="dvH")   # rowsum(H) per tile
        with tc.tile_pool(name="psB", bufs=3, space="PSUM") as psB, \
             tc.tile_pool(name="psG", bufs=1, space="PSUM") as psG:
            gx0 = const.tile([P, E + 1], BF16, tag="gx0")   # gram rows 0:128 | ones
            gx1 = const.tile([P, E + 1], BF16, tag="gx1")   # gram rows 128:256 | ones
            nc.vector.tensor_copy(gx0[:, 0:E], gsum[:, 0:E])
            nc.vector.memset(gx0[:, E:E + 1], 1.0)
            nc.vector.tensor_copy(gx1[:, P:E], gsum[:, E:E + P])
            nc.vector.memset(gx1[:, E:E + 1], 1.0)
            pgt = psG.tile([P, P], F32, tag="pgt")
            nc.tensor.transpose(pgt[:], gsum[:, P:E], id32[:])
            nc.vector.tensor_copy(gx1[:, 0:P], pgt[:])

            for k in range(nt):
                pb = psB.tile([P, E + 1], F32, tag="pb")
                ksl = slice(k * P, (k + 1) * P)
                nc.tensor.matmul(pb[:], HT0[:, ksl], gx0[:], start=True, stop=False)
                nc.tensor.matmul(pb[:], HT1[:, ksl], gx1[:], start=False, stop=True)
                sg = scr.tile([P, E], BF16, tag="sg")
                if k % 2 == 0:
                    # ACT: sum of sign(HG - 0.5) -> 2*cnt - 256
                    nc.scalar.activation(sg[:], pb[:, 0:E], AF.Sign,
                                         bias=neghalf[:], scale=1.0,
                                         accum_out=dvS[:, k:k + 1])
                    nc.vector.tensor_copy(dvH[:, k:k + 1], pb[:, E:E + 1])
                else:
                    # DVE: direct count of (HG >= 0.5)
                    nc.vector.tensor_scalar(sg[:], pb[:, 0:E], 0.5, 0.0, op0=IsGe,
                                            op1=Add, accum_out=dvS[:, k:k + 1])
                    nc.scalar.copy(dvH[:, k:k + 1], pb[:, E:E + 1])

        # ---- dv -> scale (1 + rsqrt(dv), 0-guarded) ----
        cnt = work.tile([P, nt], F32, tag="cnt")
        # even cols: cnt = 0.5*dvS + 128 ; odd cols: cnt = dvS
        nc.vector.tensor_scalar(cnt[:, 0:nt:2], dvS[:, 0:nt:2], 0.5, float(E) / 2,
                                op0=Mult, op1=Add)
        nc.vector.tensor_copy(cnt[:, 1:nt:2], dvS[:, 1:nt:2])
        dv = work.tile([P, nt], F32, tag="dv")
        nc.vector.tensor_tensor(dv[:], cnt[:], dvH[:], op=Add)
        # r0 = sqrt(dv) * recip(max(dv,1));  dv==0 -> 0
        mx = work.tile([P, nt], F32, tag="mx")
        nc.vector.tensor_scalar_max(mx[:], dv[:], 1.0)
        rc = work.tile([P, nt], F32, tag="rc")
        nc.vector.reciprocal(rc[:], mx[:])
        sq = work.tile([P, nt], F32, tag="sq")
        nc.scalar.sqrt(sq[:], dv[:])
        r0 = work.tile([P, nt], F32, tag="r0")
        nc.vector.tensor_tensor(r0[:], sq[:], rc[:], op=Mult)
        # one Newton step: r = r0 * (1.5 - 0.5*dv*r0^2)  (keeps r=0 at dv=0)
        q = work.tile([P, nt], F32, tag="q")
        nc.vector.tensor_tensor(q[:], r0[:], r0[:], op=Mult)
        nc.vector.tensor_tensor(q[:], q[:], dv[:], op=Mult)
        nc.vector.tensor_scalar(q[:], q[:], -0.5, 1.5, op0=Mult, op1=Add)
        s1p = work.tile([P, nt], F32, tag="s1p")
        nc.vector.tensor_tensor(s1p[:], r0[:], q[:], op=Mult)
        nc.vector.tensor_scalar_add(s1p[:], s1p[:], 1.0)

        # ---- final: out = (1+r) * (U @ W) + bias ----
        # U^T for the first UTPRE tiles is produced early (fills collective window)
        UTPRE = min(32, nt)
        utpre = htst.tile([P, UTPRE * IN_C], F32, tag="utpre")
        with tc.tile_pool(name="psP", bufs=2, space="PSUM") as psP:
            for s in range(UTPRE // 4):
                ut = upool.tile([P, 4, IN_C], F32, tag="u")
                nc.sync.dma_start(ut[:], U_r[s])
                pp = psP.tile([P, 4 * IN_C], F32, tag="pp")
                for j in range(4):
                    nc.tensor.transpose(pp[:, j * IN_C:(j + 1) * IN_C],
                                        ut[:, j, :], id32[:])
                sl = slice(s * 4 * IN_C, (s + 1) * 4 * IN_C)
                if s % 2 == 0:
                    nc.vector.tensor_copy(utpre[:, sl], pp[:])
                else:
                    nc.scalar.copy(utpre[:, sl], pp[:])

        with tc.tile_pool(name="psF", bufs=2, space="PSUM") as psF:
            for s in range(ns):
                if s >= UTPRE // 4:
                    ut = upool.tile([P, 4, IN_C], F32, tag="u")
                    nc.sync.dma_start(ut[:], U_r[s])
                ob = opool.tile([P, 4, OUT_C], F32, tag="o")
                for j in range(4):
                    k = 4 * s + j
                    if k < UTPRE:
                        mut_ap = utpre[:, k * IN_C:(k + 1) * IN_C]
                    else:
                        pmt = psF.tile([P, IN_C], F32, tag="pmt")
                        nc.tensor.transpose(pmt[:], ut[:, j, :], id32[:])
                        mut = mtpool.tile([P, IN_C], F32, tag="mut")
                        nc.scalar.copy(mut[:], pmt[:])
                        mut_ap = mut[:]
                    po = psF.tile([P, OUT_C], F32, tag="po")
                    nc.tensor.matmul(po[:], mut_ap, w_sb[:], start=True, stop=True)
                    ys = scr.tile([P, OUT_C], F32, tag="ys")
                    nc.scalar.mul(ys[:], po[:], s1p[:, k:k + 1])
                    nc.vector.tensor_tensor(ob[:, j, :], ys[:], bias_b[:], op=Add)
                nc.sync.dma_start(OUT_r[s], ob[:])


_CACHE = {}


def _get_program(rows=ROWS):
    if rows not in _CACHE:
        _CACHE[rows] = build_program(rows=rows)
    return _CACHE[rows]


def _make_aux():
    id16 = np.eye(P, dtype=ml_dtypes.bfloat16)
    id32 = np.eye(P, dtype=np.float32)
    return id16, id32


def kernel(H, U, weight, bias, _rows=ROWS, _trace=False):
    H = np.ascontiguousarray(H, dtype=np.float32)
    U = np.ascontiguousarray(U, dtype=np.float32)
    weight = np.ascontiguousarray(weight, dtype=np.float32)
    bias_b = np.broadcast_to(
        np.ascontiguousarray(bias, dtype=np.float32).reshape(1, OUT_C), (P, OUT_C)
    ).copy()

    nc = _get_program(_rows)
    id16, id32 = _make_aux()
    in_maps = []
    for i in range(NCORES):
        sl = slice(i * _rows, (i + 1) * _rows)
        in_maps.append({
            "H": H[sl], "U": U[sl], "W": weight, "BIASB": bias_b,
            "ID16": id16, "ID32": id32,
        })
    res = run_bass_kernel_spmd(nc, in_maps, core_ids=list(range(NCORES)),
                               trace=_trace)
    out = np.concatenate([res.results[i]["OUT"] for i in range(NCORES)], axis=0)
    if _trace:
        return out, res
    return out
